# revision 32
# baseline (speedup 1.0000x reference)
"""Trainium2 Bass kernel for CascadedNN (dense_mlp).

Math (per batch row x of dim 256):
  f  = relu(x @ W1 + b1)           # 512
  f  = relu(f @ W2 + b2)           # 256
  a_0 = f @ Wf + bf;  a_t = f @ Wc[t-1,:256] + bc[t-1]   (t = 1..63)
  p_0 = sigmoid(a_0);  p_t = max(sigmoid(a_t + w_t * p_{t-1}), p_{t-1})
  out = [p_0, ..., p_63]           # [B, 64]

Strategy: pure data parallel over 8 cores (8192 rows each), bf16 GEMMs
with fp32 PSUM accumulation, feature-major dataflow (x pre-transposed
on host). The head (first + 63 cascade steps fused into one [256, 64]
weight) runs batch-major so each psum bank holds [128 batch, 8 f x 64 t]
in scan layout.

The 63-step serial recurrence is replaced by a fixed-point iteration of
BATCHED passes (contraction factor 0.25*max|w_t| ~ 0.04, so 2-3 passes
are exact to below bf16 noise):
  pass k:  Z = A + W~ * P_shift(k-1)   (elementwise, W~ has w_0 := 0)
           S = sigmoid(Z)              (one big ACT op per chunk)
           P(k) = running-max over t   (ONE tensor_tensor_scan per
                                        chunk: state=max(mask*state, S),
                                        mask=0 at f boundaries resets
                                        the state across columns)
Each chunk's passes are emitted interleaved into the next chunk's GEMM
instruction stream, so only the last chunk's ~10us of scan work tails
the GEMM phase. PSUM evacuation (relu+bias+downcast) is rotated across
ACT/DVE/Pool so no single engine exceeds the Tensor engine's ~63us.

Batch mapping per core: row b <-> (p, f) with b = f*128 + p.
A/O layouts are [128, f, t] with t innermost.
"""

import numpy as np
import ml_dtypes
from contextlib import ExitStack

import concourse.bacc as bacc
import concourse.bass as bass
import concourse.mybir as mybir
from concourse import tile
from concourse.bass_utils import run_bass_kernel_spmd

BF16 = mybir.dt.bfloat16
F32 = mybir.dt.float32
AF = mybir.ActivationFunctionType
OP = mybir.AluOpType

B, D, H1, H2, T = 65536, 256, 512, 256, 64
NCORES = 8
BL = B // NCORES            # 8192 rows per core
FW = BL // 128              # 64 scan columns per core
# asymmetric chunks: small first (short startup x-DMA stall) and small
# last (short scan tail after the GEMM phase)
CHUNKS = [1024, 2048, 2048, 2048, 1024]
CBMAX = max(CHUNKS)
FCMAX = CBMAX // 128        # 16 scan columns in the biggest chunk

_CACHE = {}


def _build(bench_nrep=0, rev="fp1", npass=2):
    nc = bacc.Bacc("TRN2", target_bir_lowering=False, debug=False,
                   num_devices=NCORES)
    # unique per-variant dummy input: defeats NEFF/executable cache
    # collisions between structurally-different builds with identical I/O
    vtag = nc.dram_tensor(
        f"vtag_r{bench_nrep}n{npass}v{rev}", [1, 1], F32,
        kind="ExternalInput")

    xt = nc.dram_tensor("xt", [2, 128, BL], BF16, kind="ExternalInput")
    w1 = nc.dram_tensor("w1", [2, 128, H1], BF16, kind="ExternalInput")
    b1 = nc.dram_tensor("b1", [4, 128, 1], F32, kind="ExternalInput")
    w2 = nc.dram_tensor("w2", [4, 128, H2], BF16, kind="ExternalInput")
    b2 = nc.dram_tensor("b2", [2, 128, 1], F32, kind="ExternalInput")
    wcat = nc.dram_tensor("wcat", [2, 128, T], BF16, kind="ExternalInput")
    bct = nc.dram_tensor("bct", [128, 512], F32, kind="ExternalInput")
    wbt = nc.dram_tensor("wbt", [128, FCMAX * T], BF16, kind="ExternalInput")
    out = nc.dram_tensor("out", [BL, T], F32, kind="ExternalOutput")

    with tile.TileContext(nc) as tc, ExitStack() as ctx:
        wpool = ctx.enter_context(tc.tile_pool(name="wts", bufs=1))
        xpool = ctx.enter_context(tc.tile_pool(name="xin", bufs=2))
        f1pool = ctx.enter_context(tc.tile_pool(name="f1", bufs=1))
        f2pool = ctx.enter_context(tc.tile_pool(name="f2", bufs=1))
        apool = ctx.enter_context(tc.tile_pool(name="ac", bufs=1))
        opool = ctx.enter_context(tc.tile_pool(name="oc", bufs=1))
        zpool = ctx.enter_context(tc.tile_pool(name="zz", bufs=2))
        ppool = ctx.enter_context(tc.tile_pool(name="pp", bufs=2))
        pspool = ctx.enter_context(
            tc.tile_pool(name="ps", bufs=3, space=bass.MemorySpace.PSUM))

        # resident weights / constants
        w1sb = [wpool.tile([128, H1], BF16, name=f"w1_{k}", tag=f"w1_{k}")
                for k in range(2)]
        w2sb = [wpool.tile([128, H2], BF16, name=f"w2_{k}", tag=f"w2_{k}")
                for k in range(4)]
        wcsb = [wpool.tile([128, T], BF16, name=f"wc_{k}", tag=f"wc_{k}")
                for k in range(2)]
        b1sb = [wpool.tile([128, 1], F32, name=f"b1_{m}", tag=f"b1_{m}")
                for m in range(4)]
        b2sb = [wpool.tile([128, 1], F32, name=f"b2_{m}", tag=f"b2_{m}")
                for m in range(2)]
        bcsb = wpool.tile([128, 512], F32, name="bc", tag="bc")
        w3sb = wpool.tile([128, FCMAX * T], BF16, name="w3", tag="w3")
        mask = wpool.tile([128, FCMAX * T], F32, name="mk", tag="mk")
        vtsb = wpool.tile([1, 1], F32, name="vt", tag="vt")
        # weight loads stay OFF the sync queue so chunk-0's x DMA (sync)
        # starts immediately
        for k in range(2):
            nc.scalar.dma_start(w1sb[k][:], w1[k])
        for k in range(4):
            nc.scalar.dma_start(w2sb[k][:], w2[k])
            nc.scalar.dma_start(b1sb[k][:], b1[k])
        for k in range(2):
            nc.gpsimd.dma_start(wcsb[k][:], wcat[k])
            nc.gpsimd.dma_start(b2sb[k][:], b2[k])
        nc.gpsimd.dma_start(bcsb[:], bct[:])
        nc.gpsimd.dma_start(w3sb[:], wbt[:])
        nc.scalar.dma_start(vtsb[:], vtag[:])
        nc.vector.memset(mask[:], 1.0)
        mask3 = mask[:].rearrange("p (f t) -> p f t", t=T)
        nc.vector.memset(mask3[:, :, 0:1], 0.0)

        # per-chunk head preactivations A, full-core output O
        Acs = [apool.tile([128, (cb // 128) * T], BF16, name=f"A{c}",
                          tag=f"A{c}")
               for c, cb in enumerate(CHUNKS)]
        O = opool.tile([128, FW * T], F32, name="O", tag="O")
        O3 = O[:].rearrange("p (f t) -> p f t", t=T)
        ov = out[:].rearrange("(f p) t -> p f t", p=128)

        loop = tc.For_i(0, bench_nrep, 1) if bench_nrep else None
        if loop is not None:
            loop.__enter__()

        # PSUM is only reachable from DVE and ACT (GPSIMD/Pool cannot
        # access PSUM), so evacuation alternates between those two and the
        # Pool engine gets the SBUF-only scan work instead.
        def evac_relu(e, out_ap, in_ap, bias_ap):
            if e == "a":
                nc.scalar.activation(out_ap, in_ap, AF.Relu, bias=bias_ap,
                                     scale=1.0)
            else:
                nc.vector.tensor_scalar(out_ap, in_ap, bias_ap, 0.0,
                                        OP.add, OP.max)

        pend = []

        def drain(n):
            for _ in range(min(n, len(pend))):
                pend.pop(0)()

        def scan_closures(c, foff, fc):
            """Batched fixed-point passes for chunk c's fc f-columns.

            All tensors are flat [128, fc*T] with t innermost. A holds the
            head preacts PLUS 0.5*w~ (folded into bct on host), so pass 0
            is sigmoid(A) directly (implicit coupling p=0.5). Pass k's
            coupling term is (P-0.5)*w~ with P shifted one slot right;
            elements crossing an f boundary are multiplied by w~_0 = 0.
            """
            W = fc * T
            ops = []
            st = {}

            def sigma(k, src, f32out):
                def cl():
                    st[f"S{k}"] = zpool.tile(
                        [128, W], F32 if f32out else BF16,
                        name="sg", tag="sgf" if f32out else "sg")
                    nc.scalar.activation(st[f"S{k}"][:], src(),
                                         AF.Sigmoid)
                return cl

            def scan_to(k, out_ap):
                def cl():
                    nc.vector.tensor_tensor_scan(
                        out_ap, mask[:, 0:W], st[f"S{k}"][:], 0.0,
                        OP.mult, OP.max)
                return cl

            # pass 0: A already includes the p=0.5 coupling guess
            ops.append(sigma(0, lambda: Acs[c][:], False))
            for k in range(npass - 1):
                P = ppool.tile([128, W], BF16, name=f"P{k}", tag=f"P{k % 2}")
                ops.append(scan_to(k, P[:]))

                def mk_zk(k, P):
                    def cl():
                        # Z = A' + (P_shift - 0.5)*w~  (A' has +0.5*w~)
                        U = zpool.tile([128, W], BF16, name="u", tag="u")
                        nc.vector.memset(U[:, 0:1], 0.0)
                        nc.vector.scalar_tensor_tensor(
                            U[:, 1:W], P[:, 0:W - 1], 0.5, w3sb[:, 1:W],
                            OP.subtract, OP.mult)
                        st[f"Z{k + 1}"] = zpool.tile([128, W], BF16,
                                                     name="z", tag="z")
                        nc.vector.tensor_tensor(st[f"Z{k + 1}"][:], U[:],
                                                Acs[c][:], OP.add)
                    return cl
                ops.append(mk_zk(k, P))
                ops.append(sigma(k + 1, lambda k=k: st[f"Z{k + 1}"][:],
                                 k + 1 == npass - 1))
            # final pass: scan straight into O's chunk block (f32)
            ops.append(scan_to(npass - 1, O[:, foff * T:(foff + fc) * T]))

            def dma():
                nc.sync.dma_start(ov[:, foff:foff + fc, :],
                                  O3[:, foff:foff + fc, :])
            ops.append(dma)
            return ops

        ev = [0]
        sf = [0]
        offs = [sum(CHUNKS[:i]) for i in range(len(CHUNKS))]

        def x_dma(c2):
            cb2, off2 = CHUNKS[c2], offs[c2]
            t = [xpool.tile([128, CBMAX], BF16, name=f"x{k}", tag=f"x{k}")
                 for k in range(2)]
            for k in range(2):
                nc.sync.dma_start(t[k][:, 0:cb2], xt[k][:, off2:off2 + cb2])
            return t

        xsb = x_dma(0)
        for c, cb in enumerate(CHUNKS):
            nb_c = cb // 512
            fc_c = cb // 128
            off = offs[c]
            foff = off // 128

            # L1: f1[m] = relu(W1.T @ x + b1), feature-major bf16
            f1sb = [f1pool.tile([128, CBMAX], BF16, name=f"f1_{m}",
                                tag=f"f1_{m}") for m in range(4)]

            def layer(nk, wsb, insb, outsb, bsb):
                for m in range(len(outsb)):
                    pss = [pspool.tile([128, 512], F32, name="ps",
                                       tag="ps", bufs=6)
                           for _ in range(nb_c)]
                    for k in range(nk):
                        for nb in range(nb_c):
                            nc.tensor.matmul(
                                pss[nb][:], wsb[k][:, bass.ts(m, 128)],
                                insb[k][:, bass.ts(nb, 512)],
                                start=(k == 0), stop=(k == nk - 1))
                    for nb in range(nb_c):
                        evac_relu("aadad"[ev[0] % 5],
                                  outsb[m][:, bass.ts(nb, 512)],
                                  pss[nb][:], bsb[m][:])
                        ev[0] += 1
                    drain(2)

            layer(2, w1sb, xsb, f1sb, b1sb)
            if c + 1 < len(CHUNKS):
                xsb = x_dma(c + 1)  # prefetch into the other pool buffer

            # L2: f2[m2] = relu(W2.T @ f1 + b2)
            f2sb = [f2pool.tile([128, CBMAX], BF16, name=f"f2_{m}",
                                tag=f"f2_{m}") for m in range(2)]
            layer(4, w2sb, f1sb, f2sb, b2sb)

            # head, batch-major: 8 batch tiles j share one psum bank as
            # [128 batch, 8 f x 64 t]; +bct drops them into A (scan layout)
            for jg in range(fc_c // 8):
                psw = pspool.tile([128, 512], F32, name="psw", tag="psh",
                                  bufs=2)
                for j8 in range(8):
                    j = jg * 8 + j8
                    for k in range(2):
                        nc.tensor.matmul(
                            psw[:, bass.ts(j8, T)],
                            f2sb[k][:, bass.ts(j, 128)], wcsb[k][:],
                            start=(k == 0), stop=(k == 1))
                sf[0] += 1
                nc.vector.tensor_tensor(Acs[c][:, bass.ts(jg, 512)], psw[:],
                                        bcsb[:], OP.add)
                drain(2)

            pend.extend(scan_closures(c, foff, fc_c))
            off += cb
        drain(len(pend))

        if loop is not None:
            loop.__exit__(None, None, None)

    nc.compile()
    return nc


def _prep_shared(W1, b1, W2, b2, Wf, bf, Wc, bc):
    bf16 = ml_dtypes.bfloat16
    f32 = np.float32
    W1 = np.asarray(W1, f32)
    W2 = np.asarray(W2, f32)
    Wf = np.asarray(Wf, f32)
    Wc = np.asarray(Wc, f32)
    d = {}
    d["w1"] = np.ascontiguousarray(W1.astype(bf16).reshape(2, 128, H1))
    d["w2"] = np.ascontiguousarray(W2.astype(bf16).reshape(4, 128, H2))
    wcat = np.concatenate([Wf, Wc[:, :H2].T], axis=1)   # [256, 64]
    d["wcat"] = np.ascontiguousarray(wcat.astype(bf16).reshape(2, 128, T))
    d["b1"] = np.ascontiguousarray(np.asarray(b1, f32).reshape(4, 128, 1))
    d["b2"] = np.ascontiguousarray(np.asarray(b2, f32).reshape(2, 128, 1))
    bcat = np.concatenate([np.asarray(bf, f32), np.asarray(bc, f32)])
    wtld = np.concatenate([[np.float32(0)], Wc[:, H2]]).astype(f32)
    # fold the pass-0 coupling guess (p = 0.5) into the head bias
    d["bct"] = np.ascontiguousarray(
        np.tile(bcat + np.float32(0.5) * wtld, (128, 8)).astype(f32))
    d["wbt"] = np.ascontiguousarray(
        np.tile(wtld, (128, FCMAX)).astype(bf16))       # [128, 16*64]
    return d


def _core_inputs(x, shared, c):
    bf16 = ml_dtypes.bfloat16
    xs = x[c * BL:(c + 1) * BL, :]
    m = dict(shared)
    m["xt"] = np.ascontiguousarray(xs.T.astype(bf16)).reshape(2, 128, BL)
    return m


def kernel(x, W1, b1, W2, b2, Wf, bf, Wc, bc):
    if "nc" not in _CACHE:
        _CACHE["nc"] = _build()
    nc = _CACHE["nc"]

    x = np.asarray(x, np.float32)
    shared = _prep_shared(W1, b1, W2, b2, Wf, bf, Wc, bc)
    in_maps = [_core_inputs(x, shared, c) for c in range(NCORES)]

    # zero-fill any declared inputs we don't feed (e.g. the variant tag)
    pname = nc.partition_id_tensor.name if nc.partition_id_tensor else None
    for alloc in nc.m.functions[0].allocations:
        if (isinstance(alloc, mybir.MemoryLocationSet)
                and alloc.kind == "ExternalInput"):
            nm = alloc.memorylocations[0].name
            if nm != pname:
                for m in in_maps:
                    if nm not in m:
                        m[nm] = np.zeros(tuple(alloc.tensor_shape),
                                         mybir.dt.np(alloc.dtype))

    res = run_bass_kernel_spmd(nc, in_maps, list(range(NCORES)))
    outs = [np.asarray(res.results[c]["out"], np.float32)
            for c in range(NCORES)]
    return np.concatenate(outs, axis=0)
